# Initial kernel scaffold
#
"""Multi-head self-attention block on 8 trn2 NeuronCores.

Strategy: tensor-parallel over heads (16 heads -> 2 per core) for QKV+attention,
AllToAll of attention outputs, then each core runs the full output projection for
its 1/8 token shard. See bottom for the host-side kernel() entry point.
"""
import sys
sys.path.insert(0, "/opt/trn_rl_repo")

import numpy as np
import ml_dtypes

import concourse.bass as bass
import concourse.mybir as mybir
import concourse.tile as tile
from concourse import bacc
from concourse.bass_utils import run_bass_kernel_spmd
from concourse.masks import make_identity

# Problem shape (hardcoded per contract)
N, T, D, H = 4, 2048, 1024, 16
DK = D // H          # 64
NC = 8               # cores
HPC = H // NC        # 2 heads per core
NT = N * T           # 8192 tokens
SHARD = NT // NC     # 1024 tokens per core after A2A
TCH = 512            # token chunk for QKV projection matmuls
KT_PER_N = T // 128  # 16 key tiles per batch
QC_PER_N = T // 512  # 4 query chunks of 512 per batch

F32 = mybir.dt.float32
F32R = mybir.dt.float32r
BF16 = mybir.dt.bfloat16

FT = mybir.ActivationFunctionType


def build_bass():
    nc = bacc.Bacc("TRN2", target_bir_lowering=False, debug=False, num_devices=NC)

    zT = nc.dram_tensor("zT", [D, NT], F32R, kind="ExternalInput")
    wq = nc.dram_tensor("wq", [D, HPC * DK], F32R, kind="ExternalInput")
    wk = nc.dram_tensor("wk", [D, HPC * DK], F32R, kind="ExternalInput")
    wv = nc.dram_tensor("wv", [D, HPC * DK], F32R, kind="ExternalInput")
    wout = nc.dram_tensor("wout", [D, D], F32R, kind="ExternalInput")
    masks = nc.dram_tensor("masks", [4, 128, 512], BF16, kind="ExternalInput")
    outT = nc.dram_tensor("outT", [D, SHARD], F32, kind="ExternalOutput")

    zT_v = zT.rearrange("(c p) t -> p c t", p=128)     # [128, 8, NT]
    wq_v = wq.rearrange("(c p) m -> p c m", p=128)     # [128, 8, 128]
    wk_v = wk.rearrange("(c p) m -> p c m", p=128)
    wv_v = wv.rearrange("(c p) m -> p c m", p=128)
    wout_v = wout.rearrange("(c p) m -> p c m", p=128)  # [128, 8, 1024]

    with tile.TileContext(nc) as tc:
        _build_body(nc, tc, zT_v, wq_v, wk_v, wv_v, wout_v, masks, outT)
    nc.compile()
    return nc


def _build_body(nc, tc, zT_v, wq_v, wk_v, wv_v, wout_v, masks, outT):
    import contextlib
    ctx = contextlib.ExitStack()
    with ctx:
        consts = ctx.enter_context(tc.tile_pool(name="consts", bufs=1))
        zpool = ctx.enter_context(tc.tile_pool(name="zpool", bufs=2))
        qkpool = ctx.enter_context(tc.tile_pool(name="qkpool", bufs=2))
        vpool = ctx.enter_context(tc.tile_pool(name="vpool", bufs=2))
        vtpool = ctx.enter_context(tc.tile_pool(name="vtpool", bufs=2))
        expool = ctx.enter_context(tc.tile_pool(name="expool", bufs=2))
        smalls = ctx.enter_context(tc.tile_pool(name="smalls", bufs=4))
        outpool = ctx.enter_context(tc.tile_pool(name="outpool", bufs=2))
        # PSUM pools: 4 (scores) + 2 (av) + 2 (mm) = 8 banks
        ps_sc = ctx.enter_context(tc.tile_pool(name="ps_sc", bufs=1, space="PSUM"))
        ps_av = ctx.enter_context(tc.tile_pool(name="ps_av", bufs=1, space="PSUM"))
        ps_mm = ctx.enter_context(tc.tile_pool(name="ps_mm", bufs=2, space="PSUM"))
        dram = ctx.enter_context(tc.tile_pool(name="dram", bufs=1, space="DRAM"))

        # ---- constants ----
        wq_sb = consts.tile([128, 8, 128], F32R, tag="wq")
        wk_sb = consts.tile([128, 8, 128], F32R, tag="wk")
        wv_sb = consts.tile([128, 8, 128], F32R, tag="wv")
        nc.sync.dma_start(out=wq_sb, in_=wq_v)
        nc.sync.dma_start(out=wk_sb, in_=wk_v)
        nc.sync.dma_start(out=wv_sb, in_=wv_v)
        wout_sb = consts.tile([128, 8, 1024], F32R, tag="wout")
        nc.gpsimd.dma_start(out=wout_sb, in_=wout_v)
        masks_sb = consts.tile([128, 4, 512], BF16, tag="masks")
        nc.gpsimd.dma_start(out=masks_sb, in_=masks.rearrange("d p m -> p d m"))
        ident_sb = consts.tile([128, 128], BF16, tag="ident")
        make_identity(nc, ident_sb)
        # attention output (transposed): rows = 2 local heads x 64, cols = all tokens
        attnT = consts.tile([128, NT], BF16, tag="attnT")


        pending = []

        def _proj_consume(items):
            # items: list of (half_index g, a2aout [NC, 128, 128]) — 1 or 2
            nh = len(items)
            w = 128 * nh
            rhs_bf = smalls.tile([128, NC, nh, 128], BF16, tag="rhs_bf", bufs=2,
                                 name="rhsbf")
            for i in range(NC):
                for s, (_, a2aout) in enumerate(items):
                    nc.gpsimd.dma_start(out=rhs_bf[:, i, s, :], in_=a2aout[i])
            rhs_f = smalls.tile([128, NC, nh, 128], F32R, tag="rhs_f", bufs=2,
                                name="rhsf")
            nc.vector.tensor_copy(rhs_f, rhs_bf)
            for ot in range(8):
                ps = ps_mm.tile([128, 512], F32, tag="mm", name="psproj")
                for i in range(NC):
                    nc.tensor.matmul(
                        ps[:, :w],
                        lhsT=wout_sb[:, i, ot * 128:(ot + 1) * 128],
                        rhs=rhs_f[:, i, :, :], start=(i == 0), stop=(i == NC - 1))
                ob = outpool.tile([128, 256], F32, tag="ob")
                nc.vector.tensor_copy(ob[:, :w], ps[:, :w])
                for s, (g, _) in enumerate(items):
                    nc.sync.dma_start(
                        out=outT[ot * 128:(ot + 1) * 128, g * 128:(g + 1) * 128],
                        in_=ob[:, s * 128:(s + 1) * 128])

        qkv_state = {}

        def _qkv_start(n):
            tok0 = n * T
            qt = qkpool.tile([128, T], F32R, tag="qt", name=f"qt{n}")
            kt_sb = qkpool.tile([128, T], F32R, tag="kt", name=f"kt{n}")
            vsb = vpool.tile([128, KT_PER_N, HPC, 65], BF16, tag="v",
                             name=f"v{n}")
            nc.vector.memset(vsb[:, :, :, 64:65], 1.0)
            qkv_state[n] = (qt, kt_sb, vsb)

        def _qkv_chunk(n, tci):
            tok0 = n * T
            qt, kt_sb, vsb = qkv_state[n]
            zch = zpool.tile([128, 8, TCH], F32R, tag="z", name="zch")
            nc.sync.dma_start(
                out=zch, in_=zT_v[:, :, tok0 + tci * TCH: tok0 + (tci + 1) * TCH])
            for w_sb, dst in ((wq_sb, qt), (wk_sb, kt_sb)):
                ps = ps_mm.tile([128, 512], F32, tag="mm", name="psqk")
                for dc in range(8):
                    nc.tensor.matmul(
                        ps[:, :TCH], lhsT=w_sb[:, dc, :], rhs=zch[:, dc, :],
                        start=(dc == 0), stop=(dc == 7))
                nc.vector.tensor_copy(dst[:, tci * TCH:(tci + 1) * TCH],
                                      ps[:, :TCH])
            ps = ps_mm.tile([128, 512], F32, tag="mm", name="psv")
            for dc in range(8):
                nc.tensor.matmul(
                    ps[:, :TCH], lhsT=wv_sb[:, dc, :], rhs=zch[:, dc, :],
                    start=(dc == 0), stop=(dc == 7))
            vt_bf = vtpool.tile([128, TCH], BF16, tag="vt", name="vtbf")
            nc.vector.tensor_copy(vt_bf, ps[:, :TCH])
            for sub in range(TCH // 128):
                kt_idx = (tci * TCH) // 128 + sub
                for h in range(HPC):
                    pst = ps_mm.tile([128, 64], BF16, tag="mm", name="pst")
                    nc.tensor.transpose(
                        pst, vt_bf[h * 64:(h + 1) * 64, sub * 128:(sub + 1) * 128],
                        ident_sb[h * 64:(h + 1) * 64, h * 64:(h + 1) * 64])
                    nc.vector.tensor_copy(vsb[:, kt_idx, h, 0:64], pst)

        def _attn_qc(n, qc):
            tok0 = n * T
            qt, kt_sb, vsb = qkv_state[n]
            q0 = qc * 512
            n_kt = 4 * qc + 4
            av = [ps_av.tile([65, 512], F32, tag=f"av{h}", name=f"av{h}")
                  for h in range(HPC)]
            for kt in range(n_kt):
                # columns [0, s) of this kt row-block are fully causal-masked
                d = kt - 4 * qc
                s = 128 * d if d > 0 else 0
                if 512 - s == 128:
                    s = 256  # f32r below N=256 runs 4 cyc/row; keep N>=256
                sc = ps_sc.tile([128, 2, 512], F32, tag="sc", name="sc", bufs=2)
                for h in range(HPC):
                    nc.tensor.matmul(
                        sc[:, h, s:],
                        lhsT=kt_sb[h * 64:(h + 1) * 64,
                                   kt * 128:(kt + 1) * 128],
                        rhs=qt[h * 64:(h + 1) * 64, q0 + s:q0 + 512],
                        start=True, stop=True)
                ex = expool.tile([128, 2, 512], BF16, tag="ex", name="ex",
                                 bufs=3)
                nc.scalar.activation(ex[:, :, s:], sc[:, :, s:], FT.Exp)
                if s > 0:
                    nc.vector.memset(ex[:, :, :s], 0.0)
                if d >= 0:
                    for h in range(HPC):
                        nc.vector.tensor_mul(
                            ex[:, h, s:], ex[:, h, s:], masks_sb[:, d, s:])
                for h in range(HPC):
                    nc.tensor.matmul(
                        av[h][:, s:], lhsT=vsb[:, kt, h, :], rhs=ex[:, h, s:],
                        start=(kt == 0), stop=(kt == n_kt - 1))
            for h in range(HPC):
                av_sb = smalls.tile([64, 512], F32, tag=f"avs{h}",
                                    name=f"avs{h}", bufs=2)
                nc.vector.tensor_copy(av_sb, av[h][0:64, :])
                d_sb = smalls.tile([1, 512], F32, tag="d_sb", bufs=2, name="dsb")
                nc.vector.tensor_copy(d_sb, av[h][64:65, :])
                rd = smalls.tile([1, 512], F32, tag="rd", bufs=2, name="rd")
                nc.vector.reciprocal_approx_fast(rd, d_sb)
                rdb = smalls.tile([64, 512], F32, tag="rdb", bufs=2, name="rdb")
                dscr = dram.tile([1, 512], F32, tag="dscr", bufs=2, name="dscr")
                nc.sync.dma_start(out=dscr, in_=rd)
                nc.sync.dma_start(out=rdb, in_=dscr.to_broadcast([64, 512]))
                nc.vector.tensor_mul(
                    attnT[h * 64:(h + 1) * 64, tok0 + q0: tok0 + q0 + 512],
                    av_sb, rdb)

        def _a2a_issue(g):
            # half-batch g covers tokens [g*1024, (g+1)*1024); core i owns
            # 128 tokens at offset i*128 within it
            base = g * 1024
            a2ain = dram.tile([NC, 128, 128], BF16, tag="a2ain", bufs=4,
                              name=f"a2ain{g}")
            a2aout = dram.tile([NC, 128, 128], BF16, tag="a2aout", bufs=4,
                               name=f"a2aout{g}")
            for i in range(NC):
                nc.sync.dma_start(
                    out=a2ain[i],
                    in_=attnT[:, base + i * 128: base + (i + 1) * 128])
            nc.gpsimd.collective_compute(
                "AllToAll", mybir.AluOpType.bypass,
                replica_groups=[list(range(NC))],
                ins=[a2ain.opt()], outs=[a2aout.opt()])
            pending.append((g, a2aout))

        _qkv_start(0)
        for tci in range(T // TCH):
            _qkv_chunk(0, tci)
        for n in range(N):
            if n + 1 < N:
                _qkv_start(n + 1)
            for qc in range(QC_PER_N):
                _attn_qc(n, qc)
                if n + 1 < N:
                    _qkv_chunk(n + 1, qc)
                if qc == 1 or qc == 3:
                    _a2a_issue(2 * n + qc // 2)
                elif len(pending) >= 4:
                    _proj_consume([pending.pop(0), pending.pop(0)])
        while pending:
            items = [pending.pop(0)]
            if pending:
                items.append(pending.pop(0))
            _proj_consume(items)


_NC_CACHE = None


def _get_nc():
    global _NC_CACHE
    if _NC_CACHE is None:
        _NC_CACHE = build_bass()
    return _NC_CACHE


def _make_masks():
    r = np.arange(128)[:, None]
    c = np.arange(512)[None, :]
    m = np.stack([(c >= 128 * d + r) for d in range(4)]).astype(np.float32)
    return m.astype(ml_dtypes.bfloat16)


def _prepare_in_maps(z, Wqkv, Wout):
    zT = np.ascontiguousarray(z.reshape(NT, D).T).astype(np.float32)
    scale = DK ** -0.5
    Wq = (Wqkv[:, :D] * scale).reshape(D, H, DK)
    Wk = Wqkv[:, D:2 * D].reshape(D, H, DK)
    Wv = Wqkv[:, 2 * D:].reshape(D, H, DK)
    masks = _make_masks()
    in_maps = []
    for core in range(NC):
        h0 = HPC * core
        wq_c = np.ascontiguousarray(
            Wq[:, h0:h0 + HPC, :].reshape(D, HPC * DK)).astype(np.float32)
        wk_c = np.ascontiguousarray(
            Wk[:, h0:h0 + HPC, :].reshape(D, HPC * DK)).astype(np.float32)
        wv_c = np.ascontiguousarray(
            Wv[:, h0:h0 + HPC, :].reshape(D, HPC * DK)).astype(np.float32)
        in_maps.append({
            "zT": zT, "wq": wq_c, "wk": wk_c, "wv": wv_c,
            "wout": np.ascontiguousarray(Wout).astype(np.float32),
            "masks": masks,
        })
    return in_maps


def _run(z, Wqkv, Wout, trace=False):
    nc = _get_nc()
    in_maps = _prepare_in_maps(z, Wqkv, Wout)
    res = run_bass_kernel_spmd(nc, in_maps, core_ids=list(range(NC)), trace=trace)
    out = np.empty((NT, D), dtype=np.float32)
    for core in range(NC):
        shard = res.results[core]["outT"].reshape(D, NT // 1024, 128)
        for g in range(NT // 1024):
            s0 = g * 1024 + core * 128
            out[s0:s0 + 128, :] = shard[:, g, :].T
    return out.reshape(N, T, D), res


def kernel(z, Wqkv, Wout):
    out, _ = _run(np.asarray(z), np.asarray(Wqkv), np.asarray(Wout))
    return out



# revision 31
# speedup vs baseline: 1.3910x; 1.3910x over previous
"""Multi-head self-attention block on 8 trn2 NeuronCores.

Strategy: tensor-parallel over heads (16 heads -> 2 per core). Each core
computes QKV + attention for its 2 heads over ALL tokens, then a PARTIAL
output projection (contracting only its 128 head-dims of Wout) for all
tokens. The 8 partial outputs are summed on the host — no collective.
"""
import sys
sys.path.insert(0, "/opt/trn_rl_repo")

import numpy as np
import ml_dtypes

import concourse.bass as bass
import concourse.mybir as mybir
import concourse.tile as tile
from concourse import bacc
from concourse import library_config
from concourse.bass_utils import run_bass_kernel_spmd
from concourse.masks import make_identity

# Problem shape (hardcoded per contract)
N, T, D, H = 4, 2048, 1024, 16
DK = D // H          # 64
NC = 8               # cores
HPC = H // NC        # 2 heads per core
NT = N * T           # 8192 tokens
TCH = 512            # token chunk for QKV projection matmuls
KT_PER_N = T // 128  # 16 key tiles per batch
QC_PER_N = T // 512  # 4 query chunks of 512 per batch

F32 = mybir.dt.float32
F32R = mybir.dt.float32r
BF16 = mybir.dt.bfloat16

FT = mybir.ActivationFunctionType


def build_bass():
    nc = bacc.Bacc("TRN2", target_bir_lowering=False, debug=False, num_devices=NC)

    zT = nc.dram_tensor("zT", [D, NT], BF16, kind="ExternalInput")
    wq = nc.dram_tensor("wq", [D, HPC * DK], BF16, kind="ExternalInput")
    wk = nc.dram_tensor("wk", [D, HPC * DK], BF16, kind="ExternalInput")
    wv = nc.dram_tensor("wv", [D, HPC * DK], BF16, kind="ExternalInput")
    woutc = nc.dram_tensor("woutc", [HPC * DK, D], BF16, kind="ExternalInput")
    tri = nc.dram_tensor("tri", [128, 128], BF16, kind="ExternalInput")
    outT = nc.dram_tensor("outT", [8, 128, NT], BF16, kind="ExternalOutput")
    dbg = {}
    if _DBG:
        dbg["attnT"] = nc.dram_tensor("dbg_attnT", [128, NT], BF16,
                                      kind="ExternalOutput")
        dbg["avf"] = nc.dram_tensor("dbg_avf", [65, 512], F32,
                                    kind="ExternalOutput")
        dbg["rdb"] = nc.dram_tensor("dbg_rdb", [64, 512], F32,
                                    kind="ExternalOutput")
        dbg["qt"] = nc.dram_tensor("dbg_qt", [128, T], BF16,
                                   kind="ExternalOutput")
        dbg["kt"] = nc.dram_tensor("dbg_kt", [128, T], BF16,
                                   kind="ExternalOutput")
        dbg["v"] = nc.dram_tensor("dbg_v", [128, KT_PER_N * HPC * 65], BF16,
                                  kind="ExternalOutput")
        dbg["rd"] = nc.dram_tensor("dbg_rd", [1, 512], F32,
                                   kind="ExternalOutput")

    zT_v = zT.rearrange("(c p) t -> p c t", p=128)     # [128, 8, NT]
    wq_v = wq.rearrange("(c p) m -> p c m", p=128)     # [128, 8, 128]
    wk_v = wk.rearrange("(c p) m -> p c m", p=128)
    wv_v = wv.rearrange("(c p) m -> p c m", p=128)
    woutc_v = woutc.rearrange("p (o m) -> p o m", o=8)  # [128, 8, 128]
    outT_v = outT.rearrange("o p t -> p o t")           # [128, 8, NT]

    with tile.TileContext(nc) as tc:
        _build_body(nc, tc, zT_v, wq_v, wk_v, wv_v, woutc_v, tri, outT_v, dbg)
    nc.compile()
    return nc


def _build_body(nc, tc, zT_v, wq_v, wk_v, wv_v, woutc_v, tri, outT_v, dbg):
    import contextlib
    ctx = contextlib.ExitStack()
    with ctx:
        consts = ctx.enter_context(tc.tile_pool(name="consts", bufs=1))
        zpool = ctx.enter_context(tc.tile_pool(name="zpool", bufs=3))
        qkpool = ctx.enter_context(tc.tile_pool(name="qkpool", bufs=2))
        vpool = ctx.enter_context(tc.tile_pool(name="vpool", bufs=2))
        vtpool = ctx.enter_context(tc.tile_pool(name="vtpool", bufs=2))
        expool = ctx.enter_context(tc.tile_pool(name="expool", bufs=2))
        smalls = ctx.enter_context(tc.tile_pool(name="smalls", bufs=4))
        outpool = ctx.enter_context(tc.tile_pool(name="outpool", bufs=2))
        # PSUM: 4 (scores) + 2 (av) + 2 (mm) = 8 banks
        ps_sc = ctx.enter_context(tc.tile_pool(name="ps_sc", bufs=1, space="PSUM"))
        ps_av = ctx.enter_context(tc.tile_pool(name="ps_av", bufs=1, space="PSUM"))
        ps_mm = ctx.enter_context(tc.tile_pool(name="ps_mm", bufs=2, space="PSUM"))
        dram = ctx.enter_context(tc.tile_pool(name="dram", bufs=1, space="DRAM"))

        # gpsimd library for partition_broadcast + tensor_tensor
        nc.gpsimd.load_library(library_config.proxy)

        # ---- constants (issue across queues; z chunk 0 competes only
        # with the small weights for DMA bandwidth) ----
        wq_sb = consts.tile([128, 8, 128], BF16, tag="wq")
        wk_sb = consts.tile([128, 8, 128], BF16, tag="wk")
        wv_sb = consts.tile([128, 8, 128], BF16, tag="wv")
        nc.sync.dma_start(out=wq_sb, in_=wq_v)
        nc.scalar.dma_start(out=wk_sb, in_=wk_v)
        nc.scalar.dma_start(out=wv_sb, in_=wv_v)
        tri_sb = consts.tile([128, 128], BF16, tag="tri")
        nc.gpsimd.dma_start(out=tri_sb, in_=tri[:, :])
        wout_sb = consts.tile([128, 8, 128], BF16, tag="wout")
        nc.gpsimd.dma_start(out=wout_sb, in_=woutc_v)
        ident_sb = consts.tile([128, 128], BF16, tag="ident")
        make_identity(nc, ident_sb)
        ones_row = consts.tile([1, 64], BF16, tag="ones_row")
        nc.vector.memset(ones_row, 1.0)
        # attention output (transposed): rows = 2 local heads x 64, cols = tokens
        attnT = consts.tile([128, NT], BF16, tag="attnT")

        qkv_state = {}

        def _qkv_start(n):
            qt = qkpool.tile([128, T], BF16, tag="qt", name=f"qt{n}")
            kt_sb = qkpool.tile([128, T], BF16, tag="kt", name=f"kt{n}")
            vsb = vpool.tile([128, KT_PER_N, HPC, 65], BF16, tag="v",
                             name=f"v{n}")
            nc.vector.memset(vsb[:, :, :, 64:65], 1.0)
            qkv_state[n] = (qt, kt_sb, vsb)

        def _qkv_chunk(n, tci, split=False):
            tok0 = n * T
            qt, kt_sb, vsb = qkv_state[n]
            zch = zpool.tile([128, 8, TCH], BF16, tag="z", name="zch")
            src = zT_v[:, :, tok0 + tci * TCH: tok0 + (tci + 1) * TCH]
            if split:
                for dc in range(8):
                    nc.sync.dma_start(out=zch[:, dc, :], in_=src[:, dc, :])
            else:
                nc.sync.dma_start(out=zch, in_=src)
            for w_sb, dst in ((wq_sb, qt), (wk_sb, kt_sb)):
                ps = ps_mm.tile([128, 512], F32, tag="mm", name="psqk")
                for dc in range(8):
                    nc.tensor.matmul(
                        ps[:, :TCH], lhsT=w_sb[:, dc, :], rhs=zch[:, dc, :],
                        start=(dc == 0), stop=(dc == 7))
                nc.vector.tensor_copy(dst[:, tci * TCH:(tci + 1) * TCH],
                                      ps[:, :TCH])
            ps = ps_mm.tile([128, 512], F32, tag="mm", name="psv")
            for dc in range(8):
                nc.tensor.matmul(
                    ps[:, :TCH], lhsT=wv_sb[:, dc, :], rhs=zch[:, dc, :],
                    start=(dc == 0), stop=(dc == 7))
            vt_bf = vtpool.tile([128, TCH], BF16, tag="vt", name="vtbf")
            nc.vector.tensor_copy(vt_bf, ps[:, :TCH])
            for sub in range(TCH // 128):
                kt_idx = tci * 4 + sub
                pst = ps_mm.tile([128, 128], BF16, tag="mm", name="pst")
                nc.tensor.transpose(
                    pst, vt_bf[:, sub * 128:(sub + 1) * 128], ident_sb)
                for h in range(HPC):
                    nc.vector.tensor_copy(vsb[:, kt_idx, h, 0:64],
                                          pst[:, h * 64:(h + 1) * 64])

        def _attn_qc(n, qc):
            tok0 = n * T
            qt, kt_sb, vsb = qkv_state[n]
            q0 = qc * 512
            n_kt = 4 * qc + 4
            av = [ps_av.tile([65, 512], F32, tag=f"av{h}", name=f"av{h}")
                  for h in range(HPC)]

            def _issue_av(kt, ex, s):
                for h in range(HPC):
                    nc.tensor.matmul(
                        av[h][:, s:], lhsT=vsb[:, kt, h, :], rhs=ex[:, h, s:],
                        start=(kt == 0), stop=(kt == n_kt - 1))

            pend = None  # software pipeline: AV(kt) issued after scores(kt+1)
            for kt in range(n_kt):
                # columns [0, s) of this kt row-block are fully causal-masked
                d = kt - 4 * qc
                s = 128 * d if d > 0 else 0
                sc = ps_sc.tile([128, 2, 512], F32, tag="sc", name="sc", bufs=2)
                for h in range(HPC):
                    nc.tensor.matmul(
                        sc[:, h, s:],
                        lhsT=kt_sb[h * 64:(h + 1) * 64,
                                   kt * 128:(kt + 1) * 128],
                        rhs=qt[h * 64:(h + 1) * 64, q0 + s:q0 + 512],
                        start=True, stop=True)
                if pend is not None:
                    _issue_av(*pend)
                ex = expool.tile([128, 2, 512], BF16, tag="ex", name="ex",
                                 bufs=3)
                nc.scalar.activation(ex[:, :, s:], sc[:, :, s:], FT.Exp)
                if d >= 0:
                    # only the 128-col diagonal slab needs the triangle mask
                    for h in range(HPC):
                        nc.vector.tensor_mul(
                            ex[:, h, s:s + 128], ex[:, h, s:s + 128], tri_sb)
                pend = (kt, ex, s)
            _issue_av(*pend)
            avfs = []
            for h in range(HPC):
                avf = smalls.tile([65, 512], F32, tag=f"avf{h}",
                                  name=f"avf{h}", bufs=2)
                nc.vector.tensor_copy(avf, av[h])
                avfs.append(avf)
            return av, avfs

        def _attn_norm(n, qc, av, avfs):
            tok0 = n * T
            q0 = qc * 512
            last = (n == N - 1 and qc == QC_PER_N - 1)
            for h in range(HPC):
                avf = avfs[h]
                if last:
                    # low-latency path: broadcast 1/denom via a K=1 matmul
                    # back into the (already-evacuated) av PSUM bank
                    den0 = smalls.tile([1, 512], F32, tag=f"den0{h}", bufs=2,
                                       name=f"den0{h}")
                    nc.vector.tensor_copy(den0, avf[64:65, :])
                    rd = smalls.tile([1, 512], F32, tag=f"rdf{h}", bufs=2,
                                     name=f"rdf{h}")
                    nc.vector.reciprocal_approx_fast(rd, den0)
                    rd_bf = smalls.tile([1, 512], BF16, tag=f"rdbf{h}",
                                        bufs=2, name=f"rdbf{h}")
                    nc.vector.tensor_copy(rd_bf, rd)
                    nc.tensor.matmul(av[h][0:64, :], lhsT=ones_row,
                                     rhs=rd_bf, start=True, stop=True)
                    nc.vector.tensor_mul(
                        attnT[h * 64:(h + 1) * 64,
                              tok0 + q0: tok0 + q0 + 512],
                        avf[0:64, :], av[h][0:64, :])
                    continue
                dscr = dram.tile([1, 512], F32, tag=f"dscr{h}", bufs=2,
                                 name=f"dscr{h}")
                nc.sync.dma_start(out=dscr, in_=avf[64:65, :])
                denb = smalls.tile([64, 512], F32, tag=f"denb{h}", bufs=2,
                                   name=f"denb{h}")
                nc.sync.dma_start(out=denb, in_=dscr.to_broadcast([64, 512]))
                rdb = smalls.tile([64, 512], F32, tag=f"rdb{h}", bufs=2,
                                  name=f"rdb{h}")
                nc.vector.reciprocal_approx_fast(rdb, denb)
                nc.vector.tensor_mul(
                    attnT[h * 64:(h + 1) * 64, tok0 + q0: tok0 + q0 + 512],
                    avf[0:64, :], rdb)
                if dbg and n == 0 and qc == 0 and h == 0:
                    nc.sync.dma_start(out=dbg["avf"][:, :], in_=avf)
                    nc.sync.dma_start(out=dbg["rd"][:, :], in_=rdb[0:1, :])
                    nc.sync.dma_start(out=dbg["rdb"][:, :], in_=rdb)

        def _proj_chunk(n, qc):
            tok0 = n * T + qc * 512
            ob = outpool.tile([128, 8, 512], BF16, tag="ob", name="ob")
            for ot in range(8):
                ps = ps_mm.tile([128, 512], F32, tag="mm", name="psproj")
                nc.tensor.matmul(
                    ps, lhsT=wout_sb[:, ot, :], rhs=attnT[:, tok0:tok0 + 512],
                    start=True, stop=True)
                if ot % 2 == 0:
                    nc.vector.tensor_copy(ob[:, ot, :], ps)
                else:
                    nc.scalar.activation(ob[:, ot, :], ps, FT.Copy)
                if ot % 2 == 1:
                    nc.gpsimd.dma_start(
                        out=outT_v[:, ot - 1:ot + 1, tok0:tok0 + 512],
                        in_=ob[:, ot - 1:ot + 1, :])

        # ---- schedule ----
        _qkv_start(0)
        _qkv_chunk(0, 0, split=True)
        for s in range(N * QC_PER_N):
            n, qc = s // QC_PER_N, s % QC_PER_N
            if s <= 2:
                _qkv_chunk(0, s + 1)
            av, avfs = _attn_qc(n, qc)
            if 2 <= s <= 13:
                b, c = (s - 2) // 4 + 1, (s - 2) % 4
                if c == 0:
                    _qkv_start(b)
                _qkv_chunk(b, c)
            _attn_norm(n, qc, av, avfs)
            _proj_chunk(n, qc)

        if dbg:
            qt0, kt0, vsb0 = qkv_state[0]
            nc.sync.dma_start(out=dbg["attnT"][:, :], in_=attnT)
            nc.sync.dma_start(out=dbg["qt"][:, :], in_=qt0)
            nc.sync.dma_start(out=dbg["kt"][:, :], in_=kt0)
            nc.sync.dma_start(
                out=dbg["v"].rearrange("p (a b c) -> p a b c",
                                       a=KT_PER_N, b=HPC), in_=vsb0)


_NC_CACHE = None
_DBG = False


def _get_nc():
    global _NC_CACHE
    if _NC_CACHE is None:
        _NC_CACHE = build_bass()
    return _NC_CACHE


def _prepare_in_maps(z, Wqkv, Wout):
    zT = np.ascontiguousarray(z.reshape(NT, D).T).astype(ml_dtypes.bfloat16)
    scale = DK ** -0.5
    Wq = (Wqkv[:, :D] * scale).reshape(D, H, DK)
    Wk = Wqkv[:, D:2 * D].reshape(D, H, DK)
    Wv = Wqkv[:, 2 * D:].reshape(D, H, DK)
    tri = (np.arange(128)[None, :] >= np.arange(128)[:, None]).astype(
        ml_dtypes.bfloat16)
    in_maps = []
    for core in range(NC):
        h0 = HPC * core
        wq_c = np.ascontiguousarray(
            Wq[:, h0:h0 + HPC, :].reshape(D, HPC * DK)).astype(ml_dtypes.bfloat16)
        wk_c = np.ascontiguousarray(
            Wk[:, h0:h0 + HPC, :].reshape(D, HPC * DK)).astype(ml_dtypes.bfloat16)
        wv_c = np.ascontiguousarray(
            Wv[:, h0:h0 + HPC, :].reshape(D, HPC * DK)).astype(ml_dtypes.bfloat16)
        woutc = np.ascontiguousarray(
            Wout[core * 128:(core + 1) * 128, :]).astype(ml_dtypes.bfloat16)
        in_maps.append({
            "zT": zT, "wq": wq_c, "wk": wk_c, "wv": wv_c,
            "woutc": woutc, "tri": tri,
        })
    return in_maps


def _run(z, Wqkv, Wout, trace=False):
    nc = _get_nc()
    in_maps = _prepare_in_maps(z, Wqkv, Wout)
    res = run_bass_kernel_spmd(nc, in_maps, core_ids=list(range(NC)), trace=trace)
    acc = np.zeros((8, 128, NT), dtype=np.float32)
    for core in range(NC):
        acc += res.results[core]["outT"].astype(np.float32)
    out = acc.reshape(D, NT).T
    return np.ascontiguousarray(out).reshape(N, T, D), res


def kernel(z, Wqkv, Wout):
    out, _ = _run(np.asarray(z), np.asarray(Wqkv), np.asarray(Wout))
    return out
